# revision 53
# baseline (speedup 1.0000x reference)
"""Bass/Trainium2 kernel for nn_Attention_6682969112611.

Math (faithful to the buggy torch module):
    k_t   = k.reshape(b, l, c)                  # row-major reshape, NOT a transpose
    score = (q @ k_t) / sqrt(l)                 # (b, c, c)
    score = softmax(score, axis=0)              # softmax over the BATCH axis
    out   = score @ v                           # (b, c, l)

B=16, C=2048, L=64. Sharding: the c (query-row) axis of q/score/out is split
across 8 cores (256 rows each); k and v are replicated. The batch-axis softmax
needs, for every (c, c') pair, all 16 batch values - all on one core under
c-sharding => no collectives. c' is streamed in 16 chunks of 128 (the psum /
partition dim of the score tiles); mm2 accumulates over chunks in PSUM.

Engine budget per c' chunk (ACT is pacer; fast-clock ns):
  ACT   : 5 exp instrs (3x N=1024 + 2x N=512, PSUM->SBUF bf16)   ~4.6 us
  DVE   : t1 (e[0:8]+e[8:16]) + recip(bf16 out) + en=e*r         ~3.9 us
  PE    : mm1 row-tiled pairs, d-tree (8 identity matmuls summing
          t1 slots into a psum bank), mm2 col-tiled v-stationary  ~2.7 us warm
  DMA   : k chunk 0.5MB + v chunk 0.5MB                          ~2.9 us

PSUM map (16KB/partition = 8 banks x 2KB):
  0-4KB  banks 0-1: mm1 buffer A ([P,4,CB] fp32, 4-batch exp groups)
  4-6KB  bank  2  : mm1 buffer B ([P,2,CB] fp32, 2-batch exp groups)
  6-7KB  bank 3 lo: d accumulator ([P,CB] fp32) - EXCLUSIVE bank so the
                    DVE recip read never collides with a PE write
  7-8KB  bank 3 hi: padding (unused)
  8-16KB banks 4-7: mm2 acc, 8 col-tile pair tiles [P,256] fp32

mm1 row tiling: batch->partition-half h per _PETREE_GROUPS; concurrent
(T0,T8) pairs write different psum banks (fatal otherwise). B-groups put
both batches on the SAME tile so their same-bank matmuls serialize.

The batch-sum tree runs on the PE: t1 = e[0:8]+e[8:16] on DVE, then
d = sum of t1's 8 slots as 8 accumulating identity matmuls (fp32 in PSUM).

mm2 col tiling: v[c',l] stationary (64 weight cols), en streams (N=256);
batch pairs (2t,2t+1) -> partitions 0-63/64-127 of pair-tile t. Acc banks
are pre-cleared by 4 dummy start=True matmuls so every real mm2 runs
start=False (overwrite-where-clear handles chunk 0).

Software pipeline (chunk j emission): mm1/exp(j) interleaved with
en(j-2), d-tree(j-1)+recip(j-1), mm2(j-2) split around G4; t1(j) at the
end. The exp-group pattern A,B,A,A,B hides the single-buffered A-tile's
reuse latency under the B exps at the chunk boundary.

Measured on the 8-core axon TRN2 terminal (fast-clock runs; the shared
device also shows ~1.2x-slower thermal/power states run-to-run):
112.2 us max-core HW exec in this exact config (baseline before this
work: 124.7 us; the zero-matmul HAM keep-alives are worth ~2 us by
removing mid-run re-throttles), L2 relative error vs the fp32
reference 3.83e-3.
Steady-state chunk period ~5.0-5.26 us = exp work 4.70 + one exposed
mm1 latency; head ~11.5 us (7 us NEFF preamble + DMA/warmup ramp),
tail ~4 us + ~7 us semaphore-teardown postamble. Engine busy: ACT 73 us
(pacer), PE 74 us (HAM-warm for the bulk of the run; the first 2-4
chunks run throttled regardless of warmup shape), DVE 64 us, GPSIMD
idle (its SBUF port is shared with DVE's second read port under an
exclusive lock, so offloading elementwise work there is a measured net
loss). Remaining headroom is mostly fixed overhead: preamble, teardown,
cold-clock ramp, and the exp instruction-size cap (N<=1024) forced by
the 8-bank PSUM budget (mm1 A 2 + mm1 B 1 + d 1 + mm2 acc 4).
"""

import os

import numpy as np
import ml_dtypes

B, C, L = 16, 2048, 64
NCORES = 8
CB = C // NCORES  # 256 query rows per core
NJ = 16           # c' chunks of 128
P = 128

# debug bisect knobs (comma-separated): nopetree (previous architecture:
# batch-sum tree fully on DVE, even [4,4,4,4] exp groups, 2-chunk
# pipeline), oldmm2 (en-stationary non-col-tiled mm2; implies nopetree),
# nofastrecip (fp32 recip + separate bf16 cast)
_VARIANT = set(filter(None, os.environ.get("KERNEL_VARIANT", "").split(",")))

_NC_CACHE: dict = {}

# Exp-group structure: (start, kind, entries, read_slice); entries are
# (batch_offset, psum_slot, half) in EMISSION order. A-kind groups pair
# (T0,T8) into different banks; B-kind groups put both batches on one
# tile (same psum bank -> concurrent row-tile drains would be fatal, but
# same-tile matmuls serialize). read_slice = psum slots in batch order.
# Pattern A,B,A,A,B: ending on a B group lets the next chunk's first
# A-group mm1 run under the B exp (the A buffer is free after G3's exp),
# hiding the chunk-boundary latency; the one exposed mm1 latency (G2->G3,
# same A buffer) is covered by emitting d(j-1) there on the PE.
_PETREE_GROUPS = [
    (0, "A", [(0, 0, 0), (2, 2, 1), (1, 1, 0), (3, 3, 1)], (0, 4, 1)),
    (4, "B", [(0, 0, 0), (1, 1, 0)], (0, 2, 1)),
    (6, "A", [(0, 0, 0), (2, 2, 1), (1, 1, 0), (3, 3, 1)], (0, 4, 1)),
    (10, "A", [(0, 0, 0), (2, 2, 1), (1, 1, 0), (3, 3, 1)], (0, 4, 1)),
    (14, "B", [(0, 0, 1), (1, 1, 1)], (0, 2, 1)),
]
_EVEN_GROUPS = [
    (g * 4, "A", [(0, 0, 0), (2, 2, 1), (1, 1, 0), (3, 3, 1)], (0, 4, 1))
    for g in range(4)
]


def _groups(petree: bool):
    return _PETREE_GROUPS if petree else _EVEN_GROUPS


def _batch_maps(petree: bool):
    """Per-batch (partition half, m-index) from the group tables."""
    h_of_b = [None] * B
    for gstart, _, entries, _ in _groups(petree):
        for boff, _, h in entries:
            h_of_b[gstart + boff] = h
    m_of_b = [None] * B
    cnt = [0, 0]
    for b in range(B):
        m_of_b[b] = cnt[h_of_b[b]]
        cnt[h_of_b[b]] += 1
    assert cnt == [8, 8], cnt
    return h_of_b, m_of_b


def _build_nc():
    import concourse.mybir as mybir
    import concourse.tile as tile
    from concourse import bacc

    f32 = mybir.dt.float32
    bf16 = mybir.dt.bfloat16
    Exp = mybir.ActivationFunctionType.Exp
    ADD = mybir.AluOpType.add
    MUL = mybir.AluOpType.mult

    nc = bacc.Bacc(None, target_bir_lowering=False, debug=False)

    petree = "nopetree" not in _VARIANT
    oldmm2 = "oldmm2" in _VARIANT
    assert not (petree and oldmm2), "oldmm2 implies nopetree"
    groups = _groups(petree)
    h_of_b, m_of_b = _batch_maps(petree)

    # qt[p, m, cq]: p = 64*h_of_b[b] + l, m = m_of_b[b]
    qt = nc.declare_dram_parameter("qt", [P, 8, CB], bf16, isOutput=False)
    # kt[j, p, m, c']: same (p, m) mapping as qt
    kt = nc.declare_dram_parameter("kt", [NJ, P, 8, 128], bf16, isOutput=False)
    # vt[j, c', b, l]  (16, 128, 16, 64)
    vt = nc.declare_dram_parameter("vt", [NJ, P, B, L], bf16, isOutput=False)
    if petree:
        # 128x128 identity: stationary operand of the PE d-tree matmuls
        ident = nc.declare_dram_parameter("ident", [P, P], bf16, isOutput=False)
    if oldmm2:
        outd = nc.declare_dram_parameter("outd", [4, P, 4, 2, L], f32, isOutput=True)
    else:
        # outd[p, t, cq]: b = 2t + p//64, l = p%64
        outd = nc.declare_dram_parameter("outd", [P, 8, CB], f32, isOutput=True)

    with tile.TileContext(nc) as tc:
        with (
            tc.tile_pool(name="qp", bufs=1) as qp,
            tc.tile_pool(name="kp", bufs=4) as kp,
            tc.tile_pool(name="vp", bufs=4) as vp,
            tc.tile_pool(name="ep", bufs=4) as ep,
            tc.tile_pool(name="enp", bufs=3) as enp,
            tc.tile_pool(name="tp", bufs=3) as tp,
            tc.tile_pool(name="dp", bufs=3) as dp,
            tc.tile_pool(name="osp", bufs=4) as osp,
            tc.tile_pool(name="mm1pa", bufs=2 - petree, space="PSUM") as mm1pa,
            tc.tile_pool(name="mm1pb", bufs=1, space="PSUM") as mm1pb,
            tc.tile_pool(name="dpp", bufs=1, space="PSUM") as dpp,
            tc.tile_pool(name="accp", bufs=1, space="PSUM") as accp,
        ):
            qt_s = qp.tile([P, 8, CB], bf16)
            wseed = qp.tile([P, 512], bf16, name="wseed")
            nc.vector.memset(wseed[:], 0)
            # chunk-0 critical path first: k0, qt group 0, v0; the identity
            # (first needed by d(0) a chunk later) and the rest of qt after
            k_0 = kp.tile([P, 8, 128], bf16, name="k_j")
            nc.sync.dma_start(out=k_0[:], in_=kt[0])
            nc.sync.dma_start(out=qt_s[:, 0:2], in_=qt[:, 0:2])
            v_0 = vp.tile([P, B, L], bf16, name="v_j")
            nc.sync.dma_start(out=v_0[:], in_=vt[0])
            if petree:
                ident_s = qp.tile([P, P], bf16, name="ident_s")
                nc.sync.dma_start(out=ident_s[:], in_=ident[:])
            for g in range(1, 4):
                nc.sync.dma_start(
                    out=qt_s[:, 2 * g : 2 * g + 2], in_=qt[:, 2 * g : 2 * g + 2]
                )
            # PSUM tiles are bank-aligned (2KB slots), so mm1 A (2 banks),
            # mm1 B (1 bank), d (1 bank), acc (4 banks) = 8 banks exactly,
            # and no two tiles ever share a bank.
            if oldmm2:
                accs_old = [accp.tile([P, 4, 2, L], f32, name=f"acc{t}") for t in range(4)]
                accs = None
            else:
                accs = accp.tile([P, 8, CB], f32, name="acc")

            # HAM warmup: the PE self-throttles to 1.2 GHz when idle; ~3.4us
            # of dense matmuls at kernel start (hidden under the initial
            # DMAs) flips it to 2.4 GHz, and the steady-state PE duty keeps
            # it there.
            # HAM warmup: measured across nwarm in {0,6,10,16,32}: the clock
            # never unthrottles before ~27us regardless, and the warmup block
            # sits ahead of chunk 0's mm1 in the PE FIFO, delaying the first
            # exp ~2us. Default off.
            nwarm = int(os.environ.get("KERNEL_NWARM", "0"))
            if nwarm:
                wps = mm1pa.tile([P, 4, CB], f32, name="psA")
                for i in range(nwarm):
                    h = i % 2
                    nc.tensor.matmul(
                        wps[:, 2 * h : 2 * h + 2],
                        lhsT=wseed[64 * h : 64 * h + 64, :128],
                        rhs=wseed[64 * h : 64 * h + 64, :512],
                        start=True,
                        stop=True,
                    )

            if not oldmm2:
                # pre-clear the 4 acc banks: a dummy start=True matmul per
                # bank clears its has_written bits, so all real mm2 matmuls
                # use start=False (overwrite-where-clear == accumulate-from-0)
                for u in range(4):
                    nc.tensor.matmul(
                        accs[:, 2 * u, 0:1],
                        lhsT=wseed[0:64, 0:128],
                        rhs=wseed[0:64, 0:1],
                        start=True,
                        stop=False,
                        skip_group_check=True,
                    )

            def emit_mm2_pair(j, en_j, v_j, t):
                last = j == NJ - 1
                for p_ in range(2):
                    b = 2 * t + p_
                    nc.tensor.matmul(
                        accs[64 * p_ : 64 * p_ + 64, t, :],
                        lhsT=v_j[:, b],
                        rhs=en_j[:, b],
                        start=False,
                        stop=last and p_ == 1 and t % 2 == 1,
                        skip_group_check=True,
                    )

            def emit_mm2_b_old(j, en_j, v_j, b):
                acc = accs_old[b // 4]
                for h in range(2):
                    first_in_bank = j == 0 and b % 4 == 0 and h == 0
                    last_in_bank = j == NJ - 1 and b % 4 == 3 and h == 1
                    nc.tensor.matmul(
                        acc[:, b % 4, h],
                        lhsT=en_j[:, b, h * 128 : (h + 1) * 128],
                        rhs=v_j[:, b],
                        start=first_in_bank,
                        stop=last_in_bank,
                        skip_group_check=not (first_in_bank or last_in_bank),
                    )

            def emit_mm2(j, en_j, v_j):
                if oldmm2:
                    for b in range(B):
                        emit_mm2_b_old(j, en_j, v_j, b)
                else:
                    for t in range(8):
                        emit_mm2_pair(j, en_j, v_j, t)

            # HAM keep-alive: a zero-matmul (zero weights, zero rhs) that
            # accumulates +0.0 into an acc slot - numerically exact, no
            # consumer waits on it mid-kernel. Emitted into the PE's natural
            # dependency-wait windows so the activity monitor keeps the
            # 2.4 GHz clock (mid-run re-throttles cost ~1.5-3us/run).
            ka_n = int(os.environ.get("KERNEL_KEEPALIVE", "2"))
            ka_state = [0]

            def emit_keepalive(j):
                if oldmm2 or j >= NJ - 1:
                    return
                for _ in range(ka_n):
                    t = ka_state[0] % 8
                    ka_state[0] += 1
                    nc.tensor.matmul(
                        accs[:, t, :],
                        lhsT=wseed[0:64, 0:128],
                        rhs=wseed[0:64, 0:CB],
                        start=False,
                        stop=False,
                        skip_group_check=True,
                    )

            def emit_mm1_group(ps, k_j, gstart, entries):
                for boff, slot, h in entries:
                    b = gstart + boff
                    m = m_of_b[b]
                    nc.tensor.matmul(
                        ps[:, slot],
                        lhsT=k_j[64 * h : 64 * h + 64, m],
                        rhs=qt_s[64 * h : 64 * h + 64, m],
                        start=True,
                        stop=True,
                    )

            def emit_recip(d_in):
                r_b = dp.tile([P, CB], bf16, name="r_b")
                if "nofastrecip" in _VARIANT:
                    r_f = dp.tile([P, CB], f32, name="r_f")
                    nc.vector.reciprocal_approx_fast(r_f[:], d_in)
                    nc.vector.tensor_copy(out=r_b[:], in_=r_f[:])
                else:
                    # fast recip with bf16 output: compute runs in fp32 (the
                    # seed needs the INPUT's fp32 bit layout); the write path
                    # converts, saving a separate cast instruction
                    from concourse.dve_ops import (
                        RECIP_APPROX_FAST_CONSTS,
                        RECIPROCAL_APPROX_FAST,
                    )

                    c_ = RECIP_APPROX_FAST_CONSTS
                    nc.vector._custom_dve(
                        RECIPROCAL_APPROX_FAST,
                        out=r_b[:],
                        in0=d_in,
                        s0=c_["s0"],
                        s1=c_["s1"],
                        imm2=c_["imm2"],
                    )
                return r_b

            def emit_evac(j, en_j, v_j, gh):
                # en half gh -> mm2 pairs -> per-bank psum evacuation + store
                nc.vector.tensor_tensor(
                    en_j[:, 8 * gh : 8 * gh + 8],
                    e_by_j[j][:, 8 * gh : 8 * gh + 8],
                    r_by_j[j][:, None, :].to_broadcast((P, 8, CB)),
                    MUL,
                )
                for t in range(4 * gh, 4 * gh + 4):
                    emit_mm2_pair(j, en_j, v_j, t)
                    if t % 2 == 1:
                        # all copies on ScalarE: ACT is idle at the tail
                        # while DVE still runs the en halves
                        u = t // 2
                        o_s = osp.tile([P, 2, CB], f32, name="o_s")
                        nc.scalar.copy(o_s[:], accs[:, 2 * u : 2 * u + 2])
                        nc.sync.dma_start(
                            out=outd[:, 2 * u : 2 * u + 2], in_=o_s[:]
                        )

            e_by_j: dict = {}
            r_by_j: dict = {}

            def emit_d(t1_p):
                # d: accumulate t1's 8 slots on the PE (identity weights).
                # One compound matmul: rhs iterates the 8 slots, the out AP
                # broadcasts (stride 0) onto one psum tile, so has_written
                # accumulation sums the slots; lowers to 1 LDWEIGHTS + 8 MMs.
                # (a single compound matmul with a stride-0 broadcast out AP
                # fails neuronx compilation; 8 explicit matmuls it is)
                d_ps = dpp.tile([P, CB], f32, name="d_ps")
                for i in range(8):
                    nc.tensor.matmul(
                        d_ps[:],
                        lhsT=ident_s[:],
                        rhs=t1_p[:, i],
                        start=i == 0,
                        stop=i == 7,
                    )
                return d_ps

            def emit_en(jj, e_p, r_p):
                en_ = enp.tile([P, B, CB], bf16, name="en_j")
                nc.vector.tensor_tensor(
                    en_[:],
                    e_p[:],
                    r_p[:, None, :].to_broadcast((P, B, CB)),
                    MUL,
                )
                return en_

            if petree:
                state1 = None  # (j-1): (e, t1, v)
                state2 = None  # (j-2): (e, r, v)
                for j in range(NJ):
                    if j == 0:
                        k_j, v_j = k_0, v_0
                    else:
                        k_j = kp.tile([P, 8, 128], bf16, name="k_j")
                        nc.sync.dma_start(out=k_j[:], in_=kt[j])
                        v_j = vp.tile([P, B, L], bf16, name="v_j")
                        nc.sync.dma_start(out=v_j[:], in_=vt[j])
                    e_j = ep.tile([P, B, CB], bf16, name="e_j")

                    for gi, (gstart, kind, entries, rd) in enumerate(groups):
                        pool = mm1pa if kind == "A" else mm1pb
                        shape = [P, 4, CB] if kind == "A" else [P, 2, CB]
                        ps = pool.tile(shape, f32, name="ps" + kind)
                        emit_mm1_group(ps, k_j, gstart, entries)
                        n_b = len(entries)
                        nc.scalar.activation(
                            e_j[:, gstart : gstart + n_b],
                            ps[:, rd[0] : rd[1] : rd[2]],
                            Exp,
                            scale=0.125,
                        )
                        if gi in (1, 2):
                            emit_keepalive(j)
                        if gi == 1 and state2 is not None:
                            # en(j-2): fills DVE while d(j-1) runs on the PE
                            en_p2 = emit_en(j - 2, state2[0], state2[1])
                        elif gi == 2 and state1 is not None:
                            d_ps = emit_d(state1[1])  # d(j-1) in the G2->G3 gap
                            r_prev = emit_recip(d_ps[:])
                        elif gi == 3:
                            if j == NJ - 1:
                                # tail: split t1 so d(15) can start during
                                # the last exps
                                t1_j = tp.tile([P, 8, CB], bf16, name="t1")
                                nc.vector.tensor_tensor(
                                    t1_j[:, 0:4], e_j[:, 0:4], e_j[:, 8:12], ADD
                                )
                            if state2 is not None:
                                # first half of mm2(j-2); the rest after G4
                                # so G4's mm1 isn't queued behind the block
                                for t in range(4):
                                    emit_mm2_pair(j - 2, en_p2, state2[2], t)
                        elif gi == 4:
                            if state2 is not None:
                                for t in range(4, 8):
                                    emit_mm2_pair(j - 2, en_p2, state2[2], t)
                            if j == NJ - 1:
                                # hoist en(14) off the tail's DVE queue
                                en_last = emit_en(j - 1, state1[0], r_prev)

                    if j == NJ - 1:
                        nc.vector.tensor_tensor(
                            t1_j[:, 4:8], e_j[:, 4:8], e_j[:, 12:16], ADD
                        )
                    else:
                        t1_j = tp.tile([P, 8, CB], bf16, name="t1")
                        nc.vector.tensor_tensor(
                            t1_j[:], e_j[:, 0:8], e_j[:, 8:16], ADD
                        )
                    if state1 is not None:
                        state2 = (state1[0], r_prev, state1[2])
                    state1 = (e_j, t1_j, v_j)

                # tail: d(15), mm2(14), recip(15), en(15) in halves with
                # per-bank evacuation (en(14) was emitted in the loop)
                e_p, t1_p, v_p = state1
                d_ps = emit_d(t1_p)
                emit_mm2(NJ - 2, en_last, state2[2])
                r_b = emit_recip(d_ps[:])
                e_by_j[NJ - 1] = e_p
                r_by_j[NJ - 1] = r_b
                en_j = enp.tile([P, B, CB], bf16, name="en_j")
                for gh in range(2):
                    emit_evac(NJ - 1, en_j, v_p, gh)
            else:
                pending = None
                for j in range(NJ):
                    k_j = kp.tile([P, 8, 128], bf16, name="k_j")
                    nc.sync.dma_start(out=k_j[:], in_=kt[j])
                    v_j = vp.tile([P, B, L], bf16, name="v_j")
                    nc.sync.dma_start(out=v_j[:], in_=vt[j])

                    e_j = ep.tile([P, B, CB], bf16, name="e_j")
                    for gstart, kind, entries, rd in groups:
                        ps = mm1pa.tile([P, 4, CB], f32, name="psA")
                        emit_mm1_group(ps, k_j, gstart, entries)
                        nc.scalar.activation(
                            e_j[:, gstart : gstart + 4], ps[:], Exp, scale=0.125
                        )
                    if pending is not None:
                        emit_mm2(*pending)

                    t1 = tp.tile([P, 8, CB], bf16, name="t1")
                    nc.vector.tensor_tensor(t1[:], e_j[:, 0:8], e_j[:, 8:16], ADD)
                    t2 = tp.tile([P, 4, CB], bf16, name="t2")
                    nc.vector.tensor_tensor(t2[:], t1[:, 0:4], t1[:, 4:8], ADD)
                    t3 = tp.tile([P, 2, CB], bf16, name="t3")
                    nc.vector.tensor_tensor(t3[:], t2[:, 0:2], t2[:, 2:4], ADD)
                    d_f = dp.tile([P, CB], f32, name="d_f")
                    nc.vector.tensor_tensor(d_f[:], t3[:, 0], t3[:, 1], ADD)
                    r_b = emit_recip(d_f[:])

                    en_j = enp.tile([P, B, CB], bf16, name="en_j")
                    if j == NJ - 1:
                        e_by_j[j] = e_j
                        r_by_j[j] = r_b
                        if oldmm2:
                            for gh in range(2):
                                nc.vector.tensor_tensor(
                                    en_j[:, 8 * gh : 8 * gh + 8],
                                    e_j[:, 8 * gh : 8 * gh + 8],
                                    r_b[:, None, :].to_broadcast((P, 8, CB)),
                                    MUL,
                                )
                                for b in range(8 * gh, 8 * gh + 8):
                                    emit_mm2_b_old(j, en_j, v_j, b)
                                    if b % 4 == 3:
                                        t = b // 4
                                        o_s = osp.tile(
                                            [P, 4, 2, L], f32, name="o_s"
                                        )
                                        if t % 2 == 0:
                                            nc.vector.tensor_copy(
                                                out=o_s[:], in_=accs_old[t][:]
                                            )
                                        else:
                                            nc.scalar.copy(
                                                o_s[:], accs_old[t][:]
                                            )
                                        nc.sync.dma_start(
                                            out=outd[t], in_=o_s[:]
                                        )
                        else:
                            for gh in range(2):
                                emit_evac(j, en_j, v_j, gh)
                    else:
                        nc.vector.tensor_tensor(
                            en_j[:],
                            e_j[:],
                            r_b[:, None, :].to_broadcast((P, B, CB)),
                            MUL,
                        )
                        pending = (j, en_j, v_j)

    nc.compile()
    return nc


def get_nc():
    if "nc" not in _NC_CACHE:
        _NC_CACHE["nc"] = _build_nc()
    return _NC_CACHE["nc"]


def make_in_maps(q, k, v):
    q = np.asarray(q, dtype=np.float32)
    k = np.asarray(k, dtype=np.float32)
    v = np.asarray(v, dtype=np.float32)
    petree = "nopetree" not in _VARIANT
    h_of_b, m_of_b = _batch_maps(petree)

    qb = q.astype(ml_dtypes.bfloat16)
    kb = k.astype(ml_dtypes.bfloat16)
    # qt[64h+l, m, cq] = q[b, cq, l]
    qt_all = np.empty((P, 8, C), dtype=ml_dtypes.bfloat16)
    # kt[j, 64h+l, m, c'] = k_t[b, l, j*128+c'], k_t = k.reshape(B, L, C)
    k_t = kb.reshape(B, L, C)
    ktt = np.empty((NJ, P, 8, 128), dtype=ml_dtypes.bfloat16)
    for b in range(B):
        h, m = h_of_b[b], m_of_b[b]
        qt_all[64 * h : 64 * h + 64, m, :] = qb[b].T
        ktt[:, 64 * h : 64 * h + 64, m, :] = k_t[b].reshape(L, NJ, 128).transpose(
            1, 0, 2
        )
    ktt = np.ascontiguousarray(ktt)
    # v -> bf16, (c', b, l) -> (j, c'128, b, l)
    vbt = np.ascontiguousarray(
        v.astype(ml_dtypes.bfloat16).transpose(1, 0, 2)
    ).reshape(NJ, P, B, L)

    in_maps = []
    for g in range(NCORES):
        im = {
            "qt": np.ascontiguousarray(qt_all[:, :, g * CB : (g + 1) * CB]),
            "kt": ktt,
            "vt": vbt,
        }
        if petree:
            im["ident"] = np.eye(P, dtype=ml_dtypes.bfloat16)
        in_maps.append(im)
    return in_maps


def assemble_out(results):
    out = np.empty((B, C, L), dtype=np.float32)
    oldmm2 = "oldmm2" in _VARIANT
    for g in range(NCORES):
        od = np.asarray(results[g]["outd"])
        if oldmm2:
            oc = od.transpose(0, 2, 3, 1, 4).reshape(B, CB, L)
        else:
            # od[p, t, cq]: b = 2t + p//64, l = p%64
            oc = od.reshape(2, L, 8, CB).transpose(2, 0, 3, 1).reshape(B, CB, L)
        out[:, g * CB : (g + 1) * CB, :] = oc
    return out


def run(q, k, v, trace=False, trace_kwargs=None):
    """Run on 8 NeuronCores; returns (out, BassKernelResults)."""
    from concourse.bass_utils import run_bass_kernel_spmd

    nc = get_nc()
    in_maps = make_in_maps(q, k, v)
    kwargs = {}
    if trace:
        kwargs["trace"] = True
        if trace_kwargs:
            kwargs["trace_kwargs"] = trace_kwargs
    res = run_bass_kernel_spmd(nc, in_maps, core_ids=list(range(NCORES)), **kwargs)
    return assemble_out(res.results), res


def kernel(q, k, v):
    out, _ = run(q, k, v, trace=False)
    return out


# revision 55
# speedup vs baseline: 1.1615x; 1.1615x over previous
"""Bass/Trainium2 kernel for nn_Attention_6682969112611.

Math (faithful to the buggy torch module):
    k_t   = k.reshape(b, l, c)                  # row-major reshape, NOT a transpose
    score = (q @ k_t) / sqrt(l)                 # (b, c, c)
    score = softmax(score, axis=0)              # softmax over the BATCH axis
    out   = score @ v                           # (b, c, l)

B=16, C=2048, L=64. Sharding: the c (query-row) axis of q/score/out is split
across 8 cores (256 rows each); k and v are replicated. The batch-axis softmax
needs, for every (c, c') pair, all 16 batch values - all on one core under
c-sharding => no collectives. c' is streamed in 16 chunks of 128 (the psum /
partition dim of the score tiles); mm2 accumulates over chunks in PSUM.

Engine budget per c' chunk (ACT is pacer; fast-clock ns):
  ACT   : 5 exp instrs (3x N=1024 + 2x N=512, PSUM->SBUF bf16)   ~4.6 us
  DVE   : t1 (e[0:8]+e[8:16]) + recip(bf16 out) + en=e*r         ~3.9 us
  PE    : mm1 row-tiled pairs, d-tree (8 identity matmuls summing
          t1 slots into a psum bank), mm2 col-tiled v-stationary  ~2.7 us warm
  DMA   : k chunk 0.5MB + v chunk 0.5MB                          ~2.9 us

PSUM map (16KB/partition = 8 banks x 2KB):
  0-4KB  banks 0-1: mm1 buffer A ([P,4,CB] fp32, 4-batch exp groups)
  4-6KB  bank  2  : mm1 buffer B ([P,2,CB] fp32, 2-batch exp groups)
  6-7KB  bank 3 lo: d accumulator ([P,CB] fp32) - EXCLUSIVE bank so the
                    DVE recip read never collides with a PE write
  7-8KB  bank 3 hi: padding (unused)
  8-16KB banks 4-7: mm2 acc, 8 col-tile pair tiles [P,256] fp32

mm1 row tiling: batch->partition-half h per _PETREE_GROUPS; concurrent
(T0,T8) pairs write different psum banks (fatal otherwise). B-groups put
both batches on the SAME tile so their same-bank matmuls serialize.

The batch-sum tree runs on the PE: t1 = e[0:8]+e[8:16] on DVE, then
d = sum of t1's 8 slots as 8 accumulating identity matmuls (fp32 in PSUM).

mm2 col tiling: v[c',l] stationary (64 weight cols), en streams (N=256);
batch pairs (2t,2t+1) -> partitions 0-63/64-127 of pair-tile t. Acc banks
are pre-cleared by 4 dummy start=True matmuls so every real mm2 runs
start=False (overwrite-where-clear handles chunk 0).

Software pipeline (chunk j emission): mm1/exp(j) interleaved with
en(j-2), d-tree(j-1)+recip(j-1), mm2(j-2) split around G4; t1(j) at the
end. The exp-group pattern A,B,A,A,B hides the single-buffered A-tile's
reuse latency under the B exps at the chunk boundary.

Measured on the 8-core axon TRN2 terminal (fast-clock runs; the shared
device also shows ~1.2x-slower thermal/power states run-to-run):
112.2 us max-core HW exec in this exact config (baseline before this
work: 124.7 us; the zero-matmul HAM keep-alives are worth ~2 us by
removing mid-run re-throttles), L2 relative error vs the fp32
reference 3.83e-3.
Steady-state chunk period ~5.0-5.26 us = exp work 4.70 + one exposed
mm1 latency; head ~11.5 us (7 us NEFF preamble + DMA/warmup ramp),
tail ~4 us + ~7 us semaphore-teardown postamble. Engine busy: ACT 73 us
(pacer), PE 74 us (HAM-warm for the bulk of the run; the first 2-4
chunks run throttled regardless of warmup shape), DVE 64 us, GPSIMD
idle (its SBUF port is shared with DVE's second read port under an
exclusive lock, so offloading elementwise work there is a measured net
loss). Remaining headroom is mostly fixed overhead: preamble, teardown,
cold-clock ramp, and the exp instruction-size cap (N<=1024) forced by
the 8-bank PSUM budget (mm1 A 2 + mm1 B 1 + d 1 + mm2 acc 4).
"""

import os

import numpy as np
import ml_dtypes

B, C, L = 16, 2048, 64
NCORES = 8
CB = C // NCORES  # 256 query rows per core
NJ = 16           # c' chunks of 128
P = 128

# debug bisect knobs (comma-separated): nopetree (previous architecture:
# batch-sum tree fully on DVE, even [4,4,4,4] exp groups, 2-chunk
# pipeline), oldmm2 (en-stationary non-col-tiled mm2; implies nopetree),
# nofastrecip (fp32 recip + separate bf16 cast)
_VARIANT = set(filter(None, os.environ.get("KERNEL_VARIANT", "").split(",")))

_NC_CACHE: dict = {}

# Exp-group structure: (start, kind, entries, read_slice); entries are
# (batch_offset, psum_slot, half) in EMISSION order. A-kind groups pair
# (T0,T8) into different banks; B-kind groups put both batches on one
# tile (same psum bank -> concurrent row-tile drains would be fatal, but
# same-tile matmuls serialize). read_slice = psum slots in batch order.
# Pattern A,B,A,A,B: ending on a B group lets the next chunk's first
# A-group mm1 run under the B exp (the A buffer is free after G3's exp),
# hiding the chunk-boundary latency; the one exposed mm1 latency (G2->G3,
# same A buffer) is covered by emitting d(j-1) there on the PE.
_PETREE_GROUPS = [
    (0, "A", [(0, 0, 0), (2, 2, 1), (1, 1, 0), (3, 3, 1)], (0, 4, 1)),
    (4, "B", [(0, 0, 0), (1, 1, 0)], (0, 2, 1)),
    (6, "A", [(0, 0, 0), (2, 2, 1), (1, 1, 0), (3, 3, 1)], (0, 4, 1)),
    (10, "A", [(0, 0, 0), (2, 2, 1), (1, 1, 0), (3, 3, 1)], (0, 4, 1)),
    (14, "B", [(0, 0, 1), (1, 1, 1)], (0, 2, 1)),
]
_EVEN_GROUPS = [
    (g * 4, "A", [(0, 0, 0), (2, 2, 1), (1, 1, 0), (3, 3, 1)], (0, 4, 1))
    for g in range(4)
]


def _groups(petree: bool):
    return _PETREE_GROUPS if petree else _EVEN_GROUPS


def _batch_maps(petree: bool):
    """Per-batch (partition half, m-index) from the group tables."""
    h_of_b = [None] * B
    for gstart, _, entries, _ in _groups(petree):
        for boff, _, h in entries:
            h_of_b[gstart + boff] = h
    m_of_b = [None] * B
    cnt = [0, 0]
    for b in range(B):
        m_of_b[b] = cnt[h_of_b[b]]
        cnt[h_of_b[b]] += 1
    assert cnt == [8, 8], cnt
    return h_of_b, m_of_b


def _build_nc():
    import concourse.mybir as mybir
    import concourse.tile as tile
    from concourse import bacc

    f32 = mybir.dt.float32
    bf16 = mybir.dt.bfloat16
    Exp = mybir.ActivationFunctionType.Exp
    ADD = mybir.AluOpType.add
    MUL = mybir.AluOpType.mult

    nc = bacc.Bacc(None, target_bir_lowering=False, debug=False)

    petree = "nopetree" not in _VARIANT
    oldmm2 = "oldmm2" in _VARIANT
    assert not (petree and oldmm2), "oldmm2 implies nopetree"
    groups = _groups(petree)
    h_of_b, m_of_b = _batch_maps(petree)

    # qt[p, m, cq]: p = 64*h_of_b[b] + l, m = m_of_b[b]
    qt = nc.declare_dram_parameter("qt", [P, 8, CB], bf16, isOutput=False)
    # kt[j, p, m, c']: same (p, m) mapping as qt
    kt = nc.declare_dram_parameter("kt", [NJ, P, 8, 128], bf16, isOutput=False)
    # vt[j, c', b, l]  (16, 128, 16, 64)
    vt = nc.declare_dram_parameter("vt", [NJ, P, B, L], bf16, isOutput=False)
    if petree:
        # 128x128 identity: stationary operand of the PE d-tree matmuls
        ident = nc.declare_dram_parameter("ident", [P, P], bf16, isOutput=False)
    if oldmm2:
        outd = nc.declare_dram_parameter("outd", [4, P, 4, 2, L], f32, isOutput=True)
    else:
        # outd[p, t, cq]: b = 2t + p//64, l = p%64
        outd = nc.declare_dram_parameter("outd", [P, 8, CB], f32, isOutput=True)

    with tile.TileContext(nc) as tc:
        with (
            tc.tile_pool(name="qp", bufs=1) as qp,
            tc.tile_pool(name="kp", bufs=4) as kp,
            tc.tile_pool(name="vp", bufs=4) as vp,
            tc.tile_pool(name="ep", bufs=4) as ep,
            tc.tile_pool(name="enp", bufs=3) as enp,
            tc.tile_pool(name="tp", bufs=3) as tp,
            tc.tile_pool(name="dp", bufs=3) as dp,
            tc.tile_pool(name="osp", bufs=4) as osp,
            tc.tile_pool(name="mm1pa", bufs=2 - petree, space="PSUM") as mm1pa,
            tc.tile_pool(name="mm1pb", bufs=1, space="PSUM") as mm1pb,
            tc.tile_pool(name="dpp", bufs=1, space="PSUM") as dpp,
            tc.tile_pool(name="accp", bufs=1, space="PSUM") as accp,
        ):
            qt_s = qp.tile([P, 8, CB], bf16)
            wseed = qp.tile([P, 512], bf16, name="wseed")
            nc.vector.memset(wseed[:], 0)
            # chunk-0 critical path first: k0, qt group 0, v0; the identity
            # (first needed by d(0) a chunk later) and the rest of qt after
            k_0 = kp.tile([P, 8, 128], bf16, name="k_j")
            nc.sync.dma_start(out=k_0[:], in_=kt[0])
            nc.sync.dma_start(out=qt_s[:, 0:2], in_=qt[:, 0:2])
            v_0 = vp.tile([P, B, L], bf16, name="v_j")
            nc.sync.dma_start(out=v_0[:], in_=vt[0])
            if petree:
                ident_s = qp.tile([P, P], bf16, name="ident_s")
                nc.sync.dma_start(out=ident_s[:], in_=ident[:])
            for g in range(1, 4):
                nc.sync.dma_start(
                    out=qt_s[:, 2 * g : 2 * g + 2], in_=qt[:, 2 * g : 2 * g + 2]
                )
            # PSUM tiles are bank-aligned (2KB slots), so mm1 A (2 banks),
            # mm1 B (1 bank), d (1 bank), acc (4 banks) = 8 banks exactly,
            # and no two tiles ever share a bank.
            if oldmm2:
                accs_old = [accp.tile([P, 4, 2, L], f32, name=f"acc{t}") for t in range(4)]
                accs = None
            else:
                accs = accp.tile([P, 8, CB], f32, name="acc")

            # HAM warmup: the PE self-throttles to 1.2 GHz when idle; ~3.4us
            # of dense matmuls at kernel start (hidden under the initial
            # DMAs) flips it to 2.4 GHz, and the steady-state PE duty keeps
            # it there.
            # HAM warmup: measured across nwarm in {0,6,10,16,32}: the clock
            # never unthrottles before ~27us regardless, and the warmup block
            # sits ahead of chunk 0's mm1 in the PE FIFO, delaying the first
            # exp ~2us. Default off.
            nwarm = int(os.environ.get("KERNEL_NWARM", "0"))
            if nwarm:
                wps = mm1pa.tile([P, 4, CB], f32, name="psA")
                for i in range(nwarm):
                    h = i % 2
                    nc.tensor.matmul(
                        wps[:, 2 * h : 2 * h + 2],
                        lhsT=wseed[64 * h : 64 * h + 64, :128],
                        rhs=wseed[64 * h : 64 * h + 64, :512],
                        start=True,
                        stop=True,
                    )

            if not oldmm2:
                # pre-clear the 4 acc banks: a dummy start=True matmul per
                # bank clears its has_written bits, so all real mm2 matmuls
                # use start=False (overwrite-where-clear == accumulate-from-0)
                for u in range(4):
                    nc.tensor.matmul(
                        accs[:, 2 * u, 0:1],
                        lhsT=wseed[0:64, 0:128],
                        rhs=wseed[0:64, 0:1],
                        start=True,
                        stop=False,
                        skip_group_check=True,
                    )

            def emit_mm2_pair(j, en_j, v_j, t):
                last = j == NJ - 1
                for p_ in range(2):
                    b = 2 * t + p_
                    nc.tensor.matmul(
                        accs[64 * p_ : 64 * p_ + 64, t, :],
                        lhsT=v_j[:, b],
                        rhs=en_j[:, b],
                        start=False,
                        stop=last and p_ == 1 and t % 2 == 1,
                        skip_group_check=True,
                    )

            def emit_mm2_b_old(j, en_j, v_j, b):
                acc = accs_old[b // 4]
                for h in range(2):
                    first_in_bank = j == 0 and b % 4 == 0 and h == 0
                    last_in_bank = j == NJ - 1 and b % 4 == 3 and h == 1
                    nc.tensor.matmul(
                        acc[:, b % 4, h],
                        lhsT=en_j[:, b, h * 128 : (h + 1) * 128],
                        rhs=v_j[:, b],
                        start=first_in_bank,
                        stop=last_in_bank,
                        skip_group_check=not (first_in_bank or last_in_bank),
                    )

            def emit_mm2(j, en_j, v_j):
                if oldmm2:
                    for b in range(B):
                        emit_mm2_b_old(j, en_j, v_j, b)
                else:
                    for t in range(8):
                        emit_mm2_pair(j, en_j, v_j, t)

            # HAM keep-alive: a zero-matmul (zero weights, zero rhs) that
            # accumulates +0.0 into an acc slot - numerically exact, no
            # consumer waits on it mid-kernel. Emitted into the PE's natural
            # dependency-wait windows so the activity monitor keeps the
            # 2.4 GHz clock (mid-run re-throttles cost ~1.5-3us/run).
            ka_n = int(os.environ.get("KERNEL_KEEPALIVE", "2"))
            ka_state = [0]

            def emit_keepalive(j):
                if oldmm2 or j >= NJ - 1:
                    return
                for _ in range(ka_n):
                    t = ka_state[0] % 8
                    ka_state[0] += 1
                    nc.tensor.matmul(
                        accs[:, t, :],
                        lhsT=wseed[0:64, 0:128],
                        rhs=wseed[0:64, 0:CB],
                        start=False,
                        stop=False,
                        skip_group_check=True,
                    )

            def emit_mm1_group(ps, k_j, gstart, entries):
                for boff, slot, h in entries:
                    b = gstart + boff
                    m = m_of_b[b]
                    nc.tensor.matmul(
                        ps[:, slot],
                        lhsT=k_j[64 * h : 64 * h + 64, m],
                        rhs=qt_s[64 * h : 64 * h + 64, m],
                        start=True,
                        stop=True,
                    )

            def emit_recip(d_in):
                r_b = dp.tile([P, CB], bf16, name="r_b")
                if "nofastrecip" in _VARIANT:
                    r_f = dp.tile([P, CB], f32, name="r_f")
                    nc.vector.reciprocal_approx_fast(r_f[:], d_in)
                    nc.vector.tensor_copy(out=r_b[:], in_=r_f[:])
                else:
                    # fast recip with bf16 output: compute runs in fp32 (the
                    # seed needs the INPUT's fp32 bit layout); the write path
                    # converts, saving a separate cast instruction
                    from concourse.dve_ops import (
                        RECIP_APPROX_FAST_CONSTS,
                        RECIPROCAL_APPROX_FAST,
                    )

                    c_ = RECIP_APPROX_FAST_CONSTS
                    nc.vector._custom_dve(
                        RECIPROCAL_APPROX_FAST,
                        out=r_b[:],
                        in0=d_in,
                        s0=c_["s0"],
                        s1=c_["s1"],
                        imm2=c_["imm2"],
                    )
                return r_b

            def emit_evac(j, en_j, v_j, gh):
                # en half gh -> mm2 pairs -> per-bank psum evacuation + store
                nc.vector.tensor_tensor(
                    en_j[:, 8 * gh : 8 * gh + 8],
                    e_by_j[j][:, 8 * gh : 8 * gh + 8],
                    r_by_j[j][:, None, :].to_broadcast((P, 8, CB)),
                    MUL,
                )
                for t in range(4 * gh, 4 * gh + 4):
                    emit_mm2_pair(j, en_j, v_j, t)
                    if t % 2 == 1:
                        # all copies on ScalarE: ACT is idle at the tail
                        # while DVE still runs the en halves
                        u = t // 2
                        o_s = osp.tile([P, 2, CB], f32, name="o_s")
                        nc.scalar.copy(o_s[:], accs[:, 2 * u : 2 * u + 2])
                        nc.sync.dma_start(
                            out=outd[:, 2 * u : 2 * u + 2], in_=o_s[:]
                        )

            e_by_j: dict = {}
            r_by_j: dict = {}

            def emit_d(t1_p):
                # d: accumulate t1's 8 slots on the PE (identity weights).
                # One compound matmul: rhs iterates the 8 slots, the out AP
                # broadcasts (stride 0) onto one psum tile, so has_written
                # accumulation sums the slots; lowers to 1 LDWEIGHTS + 8 MMs.
                # (a single compound matmul with a stride-0 broadcast out AP
                # fails neuronx compilation; 8 explicit matmuls it is)
                d_ps = dpp.tile([P, CB], f32, name="d_ps")
                for i in range(8):
                    nc.tensor.matmul(
                        d_ps[:],
                        lhsT=ident_s[:],
                        rhs=t1_p[:, i],
                        start=i == 0,
                        stop=i == 7,
                    )
                return d_ps

            def emit_en(jj, e_p, r_p):
                en_ = enp.tile([P, B, CB], bf16, name="en_j")
                nc.vector.tensor_tensor(
                    en_[:],
                    e_p[:],
                    r_p[:, None, :].to_broadcast((P, B, CB)),
                    MUL,
                )
                return en_

            if petree:
                state1 = None  # (j-1): (e, t1, v)
                state2 = None  # (j-2): (e, r, v)
                for j in range(NJ):
                    if j == 0:
                        k_j, v_j = k_0, v_0
                    else:
                        k_j = kp.tile([P, 8, 128], bf16, name="k_j")
                        nc.sync.dma_start(out=k_j[:], in_=kt[j])
                        v_j = vp.tile([P, B, L], bf16, name="v_j")
                        nc.sync.dma_start(out=v_j[:], in_=vt[j])
                    e_j = ep.tile([P, B, CB], bf16, name="e_j")

                    for gi, (gstart, kind, entries, rd) in enumerate(groups):
                        pool = mm1pa if kind == "A" else mm1pb
                        shape = [P, 4, CB] if kind == "A" else [P, 2, CB]
                        ps = pool.tile(shape, f32, name="ps" + kind)
                        emit_mm1_group(ps, k_j, gstart, entries)
                        n_b = len(entries)
                        nc.scalar.activation(
                            e_j[:, gstart : gstart + n_b],
                            ps[:, rd[0] : rd[1] : rd[2]],
                            Exp,
                            scale=0.125,
                        )
                        if gi in (1, 2):
                            emit_keepalive(j)
                        if gi == 1 and state2 is not None:
                            # en(j-2): fills DVE while d(j-1) runs on the PE
                            en_p2 = emit_en(j - 2, state2[0], state2[1])
                        elif gi == 2 and state1 is not None:
                            d_ps = emit_d(state1[1])  # d(j-1) in the G2->G3 gap
                            r_prev = emit_recip(d_ps[:])
                        elif gi == 3:
                            if j == NJ - 1:
                                # tail: split t1 so d(15) can start during
                                # the last exps
                                t1_j = tp.tile([P, 8, CB], bf16, name="t1")
                                nc.vector.tensor_tensor(
                                    t1_j[:, 0:4], e_j[:, 0:4], e_j[:, 8:12], ADD
                                )
                            if state2 is not None:
                                # first half of mm2(j-2); the rest after G4
                                # so G4's mm1 isn't queued behind the block
                                for t in range(4):
                                    emit_mm2_pair(j - 2, en_p2, state2[2], t)
                        elif gi == 4:
                            if state2 is not None:
                                for t in range(4, 8):
                                    emit_mm2_pair(j - 2, en_p2, state2[2], t)
                            if j == NJ - 1:
                                # hoist en(14) off the tail's DVE queue
                                en_last = emit_en(j - 1, state1[0], r_prev)

                    if j == NJ - 1:
                        nc.vector.tensor_tensor(
                            t1_j[:, 4:8], e_j[:, 4:8], e_j[:, 12:16], ADD
                        )
                    else:
                        t1_j = tp.tile([P, 8, CB], bf16, name="t1")
                        nc.vector.tensor_tensor(
                            t1_j[:], e_j[:, 0:8], e_j[:, 8:16], ADD
                        )
                    if state1 is not None:
                        state2 = (state1[0], r_prev, state1[2])
                    state1 = (e_j, t1_j, v_j)

                # tail: d(15), mm2(14), recip(15), en(15) in halves with
                # per-bank evacuation (en(14) was emitted in the loop)
                e_p, t1_p, v_p = state1
                d_ps = emit_d(t1_p)
                emit_mm2(NJ - 2, en_last, state2[2])
                r_b = emit_recip(d_ps[:])
                e_by_j[NJ - 1] = e_p
                r_by_j[NJ - 1] = r_b
                en_j = enp.tile([P, B, CB], bf16, name="en_j")
                for gh in range(2):
                    emit_evac(NJ - 1, en_j, v_p, gh)
            else:
                pending = None
                for j in range(NJ):
                    k_j = kp.tile([P, 8, 128], bf16, name="k_j")
                    nc.sync.dma_start(out=k_j[:], in_=kt[j])
                    v_j = vp.tile([P, B, L], bf16, name="v_j")
                    nc.sync.dma_start(out=v_j[:], in_=vt[j])

                    e_j = ep.tile([P, B, CB], bf16, name="e_j")
                    for gstart, kind, entries, rd in groups:
                        ps = mm1pa.tile([P, 4, CB], f32, name="psA")
                        emit_mm1_group(ps, k_j, gstart, entries)
                        nc.scalar.activation(
                            e_j[:, gstart : gstart + 4], ps[:], Exp, scale=0.125
                        )
                    if pending is not None:
                        emit_mm2(*pending)

                    t1 = tp.tile([P, 8, CB], bf16, name="t1")
                    nc.vector.tensor_tensor(t1[:], e_j[:, 0:8], e_j[:, 8:16], ADD)
                    t2 = tp.tile([P, 4, CB], bf16, name="t2")
                    nc.vector.tensor_tensor(t2[:], t1[:, 0:4], t1[:, 4:8], ADD)
                    t3 = tp.tile([P, 2, CB], bf16, name="t3")
                    nc.vector.tensor_tensor(t3[:], t2[:, 0:2], t2[:, 2:4], ADD)
                    d_f = dp.tile([P, CB], f32, name="d_f")
                    nc.vector.tensor_tensor(d_f[:], t3[:, 0], t3[:, 1], ADD)
                    r_b = emit_recip(d_f[:])

                    en_j = enp.tile([P, B, CB], bf16, name="en_j")
                    if j == NJ - 1:
                        e_by_j[j] = e_j
                        r_by_j[j] = r_b
                        if oldmm2:
                            for gh in range(2):
                                nc.vector.tensor_tensor(
                                    en_j[:, 8 * gh : 8 * gh + 8],
                                    e_j[:, 8 * gh : 8 * gh + 8],
                                    r_b[:, None, :].to_broadcast((P, 8, CB)),
                                    MUL,
                                )
                                for b in range(8 * gh, 8 * gh + 8):
                                    emit_mm2_b_old(j, en_j, v_j, b)
                                    if b % 4 == 3:
                                        t = b // 4
                                        o_s = osp.tile(
                                            [P, 4, 2, L], f32, name="o_s"
                                        )
                                        if t % 2 == 0:
                                            nc.vector.tensor_copy(
                                                out=o_s[:], in_=accs_old[t][:]
                                            )
                                        else:
                                            nc.scalar.copy(
                                                o_s[:], accs_old[t][:]
                                            )
                                        nc.sync.dma_start(
                                            out=outd[t], in_=o_s[:]
                                        )
                        else:
                            for gh in range(2):
                                emit_evac(j, en_j, v_j, gh)
                    else:
                        nc.vector.tensor_tensor(
                            en_j[:],
                            e_j[:],
                            r_b[:, None, :].to_broadcast((P, B, CB)),
                            MUL,
                        )
                        pending = (j, en_j, v_j)

    nc.compile()
    return nc


def get_nc():
    if "nc" not in _NC_CACHE:
        _NC_CACHE["nc"] = _build_nc()
    return _NC_CACHE["nc"]


def make_in_maps(q, k, v):
    q = np.asarray(q, dtype=np.float32)
    k = np.asarray(k, dtype=np.float32)
    v = np.asarray(v, dtype=np.float32)
    petree = "nopetree" not in _VARIANT
    h_of_b, m_of_b = _batch_maps(petree)

    qb = q.astype(ml_dtypes.bfloat16)
    kb = k.astype(ml_dtypes.bfloat16)
    # qt[64h+l, m, cq] = q[b, cq, l]
    qt_all = np.empty((P, 8, C), dtype=ml_dtypes.bfloat16)
    # kt[j, 64h+l, m, c'] = k_t[b, l, j*128+c'], k_t = k.reshape(B, L, C)
    k_t = kb.reshape(B, L, C)
    ktt = np.empty((NJ, P, 8, 128), dtype=ml_dtypes.bfloat16)
    for b in range(B):
        h, m = h_of_b[b], m_of_b[b]
        qt_all[64 * h : 64 * h + 64, m, :] = qb[b].T
        ktt[:, 64 * h : 64 * h + 64, m, :] = k_t[b].reshape(L, NJ, 128).transpose(
            1, 0, 2
        )
    ktt = np.ascontiguousarray(ktt)
    # v -> bf16, (c', b, l) -> (j, c'128, b, l)
    vbt = np.ascontiguousarray(
        v.astype(ml_dtypes.bfloat16).transpose(1, 0, 2)
    ).reshape(NJ, P, B, L)

    in_maps = []
    for g in range(NCORES):
        im = {
            "qt": np.ascontiguousarray(qt_all[:, :, g * CB : (g + 1) * CB]),
            "kt": ktt,
            "vt": vbt,
        }
        if petree:
            im["ident"] = np.eye(P, dtype=ml_dtypes.bfloat16)
        in_maps.append(im)
    return in_maps


def assemble_out(results):
    out = np.empty((B, C, L), dtype=np.float32)
    oldmm2 = "oldmm2" in _VARIANT
    for g in range(NCORES):
        od = np.asarray(results[g]["outd"])
        if oldmm2:
            oc = od.transpose(0, 2, 3, 1, 4).reshape(B, CB, L)
        else:
            # od[p, t, cq]: b = 2t + p//64, l = p%64
            oc = od.reshape(2, L, 8, CB).transpose(2, 0, 3, 1).reshape(B, CB, L)
        out[:, g * CB : (g + 1) * CB, :] = oc
    return out


def run(q, k, v, trace=False, trace_kwargs=None):
    """Run on 8 NeuronCores; returns (out, BassKernelResults)."""
    from concourse.bass_utils import run_bass_kernel_spmd

    nc = get_nc()
    in_maps = make_in_maps(q, k, v)
    kwargs = {}
    if trace:
        kwargs["trace"] = True
        if trace_kwargs:
            kwargs["trace_kwargs"] = trace_kwargs
    res = run_bass_kernel_spmd(nc, in_maps, core_ids=list(range(NCORES)), **kwargs)
    return assemble_out(res.results), res


def kernel(q, k, v):
    out, _ = run(q, k, v, trace=False)
    return out
